# revision 17
# baseline (speedup 1.0000x reference)
"""Trainium2 Bass kernel for DerivativeRBF: K(X,X2), grad_K, hess_K.

Math (reference):
  ls = softplus(uls) (D,), var = softplus(uvar)
  Xs = X/ls, X2s = X2/ls
  K[n,m]    = var*exp(-0.25*(|Xs_n|^2 - 2 Xs_n.X2s_m + |X2s_m|^2))      (N,M)
  grad_K    rows d*N+n: -0.5*(X[n,d]-X2[m,d])/ls_d^2 * K[n,m]           (N*D,M)
  kNN[i,j]  = var*exp(-0.25*sqdist(Xs_i,Xs_j))                          (N,N)
  hess_K[a*N+i, b*N+j] = kNN[i,j]*(delta_ab*0.5/ls_a^2
                                   - 0.25*s_a[i,j]*s_b[i,j])            (N*D,N*D)
  with s_d[i,j] = (X[i,d]-X[j,d])/ls_d^2.

Sharding: rows of X split across 8 cores, 64 rows each (SPMD, no
collectives). Each core computes its block-rows of all three outputs;
the host reassembles.

Host prep (tiny, O(N*D)): softplus of the 17 hyperparameters and the
scaled/transposed operand tables below. Device does all the heavy work:
the -0.25*sqdist matmuls + exp, and the D*D grid of N_loc x N hessian
blocks (8.4M f32 per core) plus grad blocks, via fused DVE/GPSIMD ops.

Device-side layout, per core (i0 = 64*core):
  p_d[x] = X[x,d]/ls_d^2;  shat_d[i,j] := p_d[j] - p_d[i] = -s_d[i,j]
  hess block (a,b) = (shat_a * (-0.25*kNN)) * shat_b  (+ delta_ab*c_a*kNN)
  row-pair tile r < 8: partitions 0:64 -> a=2r, 64:128 -> a=2r+1
"""

import sys

if "/opt/trn_rl_repo" not in sys.path:
    sys.path.insert(0, "/opt/trn_rl_repo")

from contextlib import ExitStack

import numpy as np

import concourse.bacc as bacc
import concourse.bass as bass
import concourse.tile as tile
from concourse import mybir

F32 = mybir.dt.float32
AF = mybir.ActivationFunctionType
OP = mybir.AluOpType

N = 512          # rows of X / X2
D = 16           # feature dim
NCORES = 8
NL = N // NCORES  # 64 local rows per core
R = D // 2        # 8 row-pair tiles (two feature dims per 128-partition tile)

# All matmul operands are packed into one [18, MMW] tensor (single DMA ->
# single completion semaphore; the fp32 fused-ldweights Matmult can encode
# only one sync wait, so every matmul may depend on at most one semaphore).
# Column layout of MM18:
#   0:128     L18    rows 0:16 Xs_loc.T dup; row 16 ones; row 17 -0.25*|Xs_loc|^2 dup
#   128:640   RX18   rows 0:16 0.5*Xs.T; row 16 -0.25*|Xs_j|^2; row 17 ones
#   640:1152  RX218  same with X2s
#   1152:1280 sel2   row 0: 1 on cols 0:64; row 1: 1 on cols 64:128
#   1280:1408 onesE  row 0 all ones, row 1 zeros
#   1408:1536 onesO  row 0 zeros, row 1 all ones
#   1536:5632 P2     row 0: p_{2r}[j] r-major; row 1: p_{2r+1}[j]
#   5632:9728 Q2     same from q_d[m] = X2[m,d]/ls_d^2
MMW = 9728
C_L18, C_RX, C_RX2, C_SEL, C_ONE, C_ONO, C_P2, C_Q2 = (
    0, 128, 640, 1152, 1280, 1408, 1536, 5632)
# Per-partition scalar tables (ACT bias / DVE scalar operands) in one
# [128, 32] tensor VT:
#   0:16  PL4N[p, d] = -p_d[i0 + p%64]        (bias for S_all builds)
#   16:24 PL3N col r: -p_{2r}[i] upper half, -p_{2r+1}[i] lower half
#   24:32 CB2 col r: 0.5/ls_{2r}^2 upper, 0.5/ls_{2r+1}^2 lower
VTW = 32
DVE_B = 11  # hessian column chunks 0:DVE_B on DVE, rest on GPSIMD


def _body(ctx, tc, nc, dram):
    MM_d, VT_d, K_d, G_d, H_d = dram

    sing = ctx.enter_context(tc.tile_pool(name="sing", bufs=1))
    ps_b = ctx.enter_context(tc.tile_pool(name="ps_b", bufs=4, space="PSUM"))
    tpool = ctx.enter_context(tc.tile_pool(name="tpool", bufs=2))
    hout = ctx.enter_context(tc.tile_pool(name="hout", bufs=2))
    gout = ctx.enter_context(tc.tile_pool(name="gout", bufs=2))

    MM = sing.tile([D + 2, MMW], F32)
    nc.sync.dma_start(out=MM, in_=MM_d[:, :])
    VT = sing.tile([128, VTW], F32)
    nc.sync.dma_start(out=VT, in_=VT_d[:, :])

    L18 = MM[:, C_L18:C_L18 + 128]
    RX18 = MM[:, C_RX:C_RX + N]
    RX218 = MM[:, C_RX2:C_RX2 + N]
    sel2 = MM[0:2, C_SEL:C_SEL + 128]
    onesE = MM[0:2, C_ONE:C_ONE + 128]
    onesO = MM[0:2, C_ONO:C_ONO + 128]
    P2 = MM[0:2, C_P2:C_P2 + R * N]
    Q2 = MM[0:2, C_Q2:C_Q2 + R * N]
    PL4N = VT[:, 0:D]
    PL3N = VT[:, D:D + R]
    CB2 = VT[:, D + R:D + 2 * R]

    # ---- kNN / K: z = -0.25*sqdist via one K=18 matmul each -------------
    zX = ps_b.tile([128, N], F32, tag="pbig")
    nc.tensor.matmul(zX, L18, RX18, start=True, stop=True)
    kNN = sing.tile([128, N], F32)
    nc.scalar.activation(out=kNN, in_=zX, func=AF.Exp)
    kNNq = sing.tile([128, N], F32)  # -0.25 * kNN
    nc.scalar.activation(out=kNNq, in_=kNN, func=AF.Copy, scale=-0.25)

    zK = ps_b.tile([128, N], F32, tag="pbig")
    nc.tensor.matmul(zK, L18, RX218, start=True, stop=True)
    K_dup = sing.tile([128, N], F32)
    nc.scalar.activation(out=K_dup, in_=zK, func=AF.Exp)
    nc.sync.dma_start(out=K_d[:, :], in_=K_dup[0:NL, :])
    K05 = sing.tile([128, N], F32)  # 0.5 * K
    nc.scalar.activation(out=K05, in_=K_dup, func=AF.Copy, scale=0.5)

    # ---- S_all: shat_b[i,j] = p_b[j] - p_b[i], dup halves, b-major ------
    # built on the (otherwise idle) scalar engine: Identity(bps + (-p_b[i]))
    S_all = sing.tile([128, D * N], F32)
    for b in range(D):
        r, sel = b // 2, (onesE if b % 2 == 0 else onesO)
        bps = ps_b.tile([128, N], F32, tag="pbig")
        nc.tensor.matmul(bps, sel, P2[:, r * N:(r + 1) * N],
                         start=True, stop=True)
        nc.scalar.activation(out=S_all[:, b * N:(b + 1) * N], in_=bps,
                             func=AF.Identity, bias=PL4N[:, b:b + 1])

    # ---- main hessian loop ---------------------------------------------
    for r in range(R):
        # pb = p_{2r}[j] on the upper 64 partitions, p_{2r+1}[j] on the lower
        pb = ps_b.tile([128, N], F32, tag="pbig")
        nc.tensor.matmul(pb, sel2, P2[:, r * N:(r + 1) * N],
                         start=True, stop=True)
        # T_r = shat_a * (-0.25*kNN)   (a = 2r upper half, 2r+1 lower half)
        shat = tpool.tile([128, N], F32, tag="shat")
        nc.scalar.activation(out=shat, in_=pb, func=AF.Identity,
                             bias=PL3N[:, r:r + 1])
        T_r = tpool.tile([128, N], F32)
        nc.gpsimd.tensor_mul(T_r, shat, kNNq)

        # H_t[:, b*N:(b+1)*N] = T_r * S_all[b] for all 16 b, as one wide
        # DVE op (chunks 0:DVE_B) + one GPSIMD op (rest), broadcasting T_r
        # along the chunk axis with a stride-0 AP.
        H_t = hout.tile([128, D * N], F32)
        def tb(k):
            return bass.AP(tensor=T_r.tensor, offset=T_r.offset,
                           ap=[T_r.ap[0], [0, k], T_r.ap[1]])
        nD = DVE_B * N
        nc.vector.tensor_mul(
            H_t[:, 0:nD].rearrange("p (b j) -> p b j", b=DVE_B), tb(DVE_B),
            S_all[:, 0:nD].rearrange("p (b j) -> p b j", b=DVE_B))
        nc.gpsimd.tensor_mul(
            H_t[:, nD:D * N].rearrange("p (b j) -> p b j", b=D - DVE_B),
            tb(D - DVE_B),
            S_all[:, nD:D * N].rearrange("p (b j) -> p b j", b=D - DVE_B))
        # diagonal correction on the (a == b) blocks
        for half, b in ((0, 2 * r), (1, 2 * r + 1)):
            lo, hi = half * NL, half * NL + NL
            sl = H_t[lo:hi, b * N:(b + 1) * N]
            nc.vector.scalar_tensor_tensor(
                out=sl, in0=kNN[lo:hi, :], scalar=CB2[lo:hi, r:r + 1],
                in1=sl, op0=OP.mult, op1=OP.add)
        nc.sync.dma_start(out=H_d[r * 128:(r + 1) * 128, :], in_=H_t)

    # ---- grad_K ---------------------------------------------------------
    for tt in range(R):
        qb = ps_b.tile([128, N], F32, tag="pbig")
        nc.tensor.matmul(qb, sel2, Q2[:, tt * N:(tt + 1) * N],
                         start=True, stop=True)
        gq = gout.tile([128, N], F32, tag="gq")
        nc.scalar.activation(out=gq, in_=qb, func=AF.Identity,
                             bias=PL3N[:, tt:tt + 1])
        G_t = gout.tile([128, N], F32)
        nc.vector.tensor_mul(G_t, gq, K05)
        nc.scalar.dma_start(out=G_d[tt * 128:(tt + 1) * 128, :], in_=G_t)


def build_nc():
    nc = bacc.Bacc()
    MM_d = nc.dram_tensor("MM18", [D + 2, MMW], F32,
                          kind="ExternalInput").ap()
    VT_d = nc.dram_tensor("VT", [128, VTW], F32, kind="ExternalInput").ap()
    K_d = nc.dram_tensor("Kout", [NL, N], F32, kind="ExternalOutput").ap()
    G_d = nc.dram_tensor("Gout", [NL * D, N], F32, kind="ExternalOutput").ap()
    H_d = nc.dram_tensor("Hout", [NL * D, N * D], F32,
                         kind="ExternalOutput").ap()
    with tile.TileContext(nc) as tc:
        with ExitStack() as ctx:
            _body(ctx, tc, nc, (MM_d, VT_d, K_d, G_d, H_d))
    # Bacc lowering: splits multi-sem waits into EventSemaphore instructions
    # (walrus allows at most one sync wait per engine instruction on TRN2),
    # moves matmul waits to ldweights, allocates registers.
    nc.compile()
    return nc


_CACHE = {}


def get_nc():
    if "nc" not in _CACHE:
        _CACHE["nc"] = build_nc()
    return _CACHE["nc"]


def make_in_maps(X, X2, uls, uvar):
    """Host prep: softplus the 17 hyperparameters and pack the small operand
    tables (O(N*D) f64 math) into MM18/VT; shard local-row tables per core."""
    X = np.asarray(X, np.float64)
    X2 = np.asarray(X2, np.float64)
    uls = np.asarray(uls, np.float64)
    uvar = np.asarray(uvar, np.float64)

    ls = np.logaddexp(0.0, uls)            # softplus
    var = np.logaddexp(0.0, uvar)[0]
    linv2 = 1.0 / (ls * ls)

    Xs = X / ls
    X2s = X2 / ls
    P = X * linv2                          # (N, D): p_d[x]
    Q = X2 * linv2                         # (N, D): q_d[m]
    nX = -0.25 * np.sum(Xs * Xs, axis=1)   # (N,)
    nX2 = -0.25 * np.sum(X2s * X2s, axis=1)

    mm = np.zeros((D + 2, MMW))
    mm[0:D, C_RX:C_RX + N] = 0.5 * Xs.T
    mm[D, C_RX:C_RX + N] = nX
    mm[D + 1, C_RX:C_RX + N] = 1.0
    mm[0:D, C_RX2:C_RX2 + N] = 0.5 * X2s.T
    mm[D, C_RX2:C_RX2 + N] = nX2
    mm[D + 1, C_RX2:C_RX2 + N] = 1.0
    mm[0, C_SEL:C_SEL + 64] = 1.0
    mm[1, C_SEL + 64:C_SEL + 128] = 1.0
    mm[0, C_ONE:C_ONE + 128] = 1.0
    mm[1, C_ONO:C_ONO + 128] = 1.0
    mm[0:2, C_P2:C_P2 + R * N] = \
        P.T.reshape(R, 2, N).transpose(1, 0, 2).reshape(2, R * N)
    mm[0:2, C_Q2:C_Q2 + R * N] = \
        Q.T.reshape(R, 2, N).transpose(1, 0, 2).reshape(2, R * N)

    vt = np.empty((128, VTW))
    vt[0:64, D + R:D + 2 * R] = (0.5 * linv2)[0::2][None, :]
    vt[64:128, D + R:D + 2 * R] = (0.5 * linv2)[1::2][None, :]

    maps = []
    for c in range(NCORES):
        rows = slice(c * NL, (c + 1) * NL)
        Xl = Xs[rows]                       # (64, D)
        Pl = P[rows]                        # (64, D)
        nl = -0.25 * np.sum(Xl * Xl, axis=1)
        mmc = mm.copy()
        mmc[0:D, C_L18:C_L18 + 64] = Xl.T
        mmc[0:D, C_L18 + 64:C_L18 + 128] = Xl.T
        mmc[D, C_L18:C_L18 + 128] = 1.0
        mmc[D + 1, C_L18:C_L18 + 64] = nl + np.log(var)
        mmc[D + 1, C_L18 + 64:C_L18 + 128] = nl + np.log(var)
        vtc = vt.copy()
        vtc[0:64, 0:D] = -Pl
        vtc[64:128, 0:D] = -Pl
        vtc[0:64, D:D + R] = -Pl[:, 0::2]
        vtc[64:128, D:D + R] = -Pl[:, 1::2]
        maps.append({
            "MM18": np.ascontiguousarray(mmc, dtype=np.float32),
            "VT": np.ascontiguousarray(vtc, dtype=np.float32),
        })
    return maps


def assemble(results):
    K = np.empty((N, N), np.float32)
    G = np.empty((N * D, N), np.float32)
    H = np.empty((N * D, N * D), np.float32)
    Gr = G.reshape(D, NCORES, NL, N)
    Hr = H.reshape(D, NCORES, NL, N * D)
    for c, res in enumerate(results):
        K[c * NL:(c + 1) * NL] = res["Kout"]
        Gr[:, c] = res["Gout"].reshape(D, NL, N)
        Hr[:, c] = res["Hout"].reshape(D, NL, N * D)
    return K, G, H


def run(X, X2, uls, uvar, trace=False, **kw):
    from concourse.bass_utils import run_bass_kernel_spmd

    nc = get_nc()
    in_maps = make_in_maps(X, X2, uls, uvar)
    out = run_bass_kernel_spmd(nc, in_maps, core_ids=list(range(NCORES)),
                               trace=trace, **kw)
    return assemble(out.results), out


def kernel(X, X2, uls, uvar):
    (K, G, H), _ = run(X, X2, uls, uvar)
    return K, G, H


# revision 19
# speedup vs baseline: 1.1028x; 1.1028x over previous
"""Trainium2 Bass kernel for DerivativeRBF: K(X,X2), grad_K, hess_K.

Math (reference):
  ls = softplus(uls) (D,), var = softplus(uvar)
  Xs = X/ls, X2s = X2/ls
  K[n,m]    = var*exp(-0.25*(|Xs_n|^2 - 2 Xs_n.X2s_m + |X2s_m|^2))      (N,M)
  grad_K    rows d*N+n: -0.5*(X[n,d]-X2[m,d])/ls_d^2 * K[n,m]           (N*D,M)
  kNN[i,j]  = var*exp(-0.25*sqdist(Xs_i,Xs_j))                          (N,N)
  hess_K[a*N+i, b*N+j] = kNN[i,j]*(delta_ab*0.5/ls_a^2
                                   - 0.25*s_a[i,j]*s_b[i,j])            (N*D,N*D)
  with s_d[i,j] = (X[i,d]-X[j,d])/ls_d^2.

Sharding: rows of X split across 8 cores, 64 rows each (SPMD, no
collectives). Each core computes its block-rows of all three outputs;
the host reassembles.

Host prep (tiny, O(N*D)): softplus of the 17 hyperparameters and the
scaled/transposed operand tables below. Device does all the heavy work:
the -0.25*sqdist matmuls + exp, and the D*D grid of N_loc x N hessian
blocks (8.4M f32 per core) plus grad blocks, via fused DVE/GPSIMD ops.

Device-side layout, per core (i0 = 64*core):
  p_d[x] = X[x,d]/ls_d^2;  shat_d[i,j] := p_d[j] - p_d[i] = -s_d[i,j]
  hess block (a,b) = (shat_a * (-0.25*kNN)) * shat_b  (+ delta_ab*c_a*kNN)
  row-pair tile r < 8: partitions 0:64 -> a=2r, 64:128 -> a=2r+1
"""

import sys

if "/opt/trn_rl_repo" not in sys.path:
    sys.path.insert(0, "/opt/trn_rl_repo")

from contextlib import ExitStack

import numpy as np

import concourse.bacc as bacc
import concourse.bass as bass
import concourse.tile as tile
from concourse import mybir

F32 = mybir.dt.float32
AF = mybir.ActivationFunctionType
OP = mybir.AluOpType

N = 512          # rows of X / X2
D = 16           # feature dim
NCORES = 8
NL = N // NCORES  # 64 local rows per core
R = D // 2        # 8 row-pair tiles (two feature dims per 128-partition tile)

# All matmul operands are packed into one [18, MMW] tensor (single DMA ->
# single completion semaphore; the fp32 fused-ldweights Matmult can encode
# only one sync wait, so every matmul may depend on at most one semaphore).
# Column layout of MM18:
#   0:128     L18    rows 0:16 Xs_loc.T dup; row 16 ones; row 17 -0.25*|Xs_loc|^2 dup
#   128:640   RX18   rows 0:16 0.5*Xs.T; row 16 -0.25*|Xs_j|^2; row 17 ones
#   640:1152  RX218  same with X2s
#   1152:1280 sel2   row 0: 1 on cols 0:64; row 1: 1 on cols 64:128
#   1280:1408 onesE  row 0 all ones, row 1 zeros
#   1408:1536 onesO  row 0 zeros, row 1 all ones
#   1536:5632 P2     row 0: p_{2r}[j] r-major; row 1: p_{2r+1}[j]
#   5632:9728 Q2     same from q_d[m] = X2[m,d]/ls_d^2
MMW = 9728
C_L18, C_RX, C_RX2, C_SEL, C_ONE, C_ONO, C_P2, C_Q2 = (
    0, 128, 640, 1152, 1280, 1408, 1536, 5632)
# Per-partition scalar tables (ACT bias / DVE scalar operands) in one
# [128, 32] tensor VT:
#   0:16  PL4N[p, d] = -p_d[i0 + p%64]        (bias for S_all builds)
#   16:24 PL3 col r: p_{2r}[i] upper half, p_{2r+1}[i] lower half
#   24:32 CB2 col r: 0.5/ls_{2r}^2 upper, 0.5/ls_{2r+1}^2 lower
VTW = 32
DVE_B = 10  # hessian column chunks 0:DVE_B on DVE, rest on GPSIMD


def _body(ctx, tc, nc, dram):
    MM_d, VT_d, K_d, G_d, H_d = dram

    sing = ctx.enter_context(tc.tile_pool(name="sing", bufs=1))
    ps_b = ctx.enter_context(tc.tile_pool(name="ps_b", bufs=4, space="PSUM"))
    tpool = ctx.enter_context(tc.tile_pool(name="tpool", bufs=2))
    hout = ctx.enter_context(tc.tile_pool(name="hout", bufs=2))
    gout = ctx.enter_context(tc.tile_pool(name="gout", bufs=2))

    MM = sing.tile([D + 2, MMW], F32)
    nc.sync.dma_start(out=MM, in_=MM_d[:, :])
    VT = sing.tile([128, VTW], F32)
    nc.sync.dma_start(out=VT, in_=VT_d[:, :])

    L18 = MM[:, C_L18:C_L18 + 128]
    RX18 = MM[:, C_RX:C_RX + N]
    RX218 = MM[:, C_RX2:C_RX2 + N]
    sel2 = MM[0:2, C_SEL:C_SEL + 128]
    onesE = MM[0:2, C_ONE:C_ONE + 128]
    onesO = MM[0:2, C_ONO:C_ONO + 128]
    P2 = MM[0:2, C_P2:C_P2 + R * N]
    Q2 = MM[0:2, C_Q2:C_Q2 + R * N]
    PL4N = VT[:, 0:D]
    PL3 = VT[:, D:D + R]
    CB2 = VT[:, D + R:D + 2 * R]

    # ---- kNN / K: z = -0.25*sqdist via one K=18 matmul each -------------
    zX = ps_b.tile([128, N], F32, tag="pbig")
    nc.tensor.matmul(zX, L18, RX18, start=True, stop=True)
    kNN = sing.tile([128, N], F32)
    nc.scalar.activation(out=kNN, in_=zX, func=AF.Exp)
    kNNq = sing.tile([128, N], F32)  # -0.25 * kNN
    nc.scalar.activation(out=kNNq, in_=kNN, func=AF.Copy, scale=-0.25)

    zK = ps_b.tile([128, N], F32, tag="pbig")
    nc.tensor.matmul(zK, L18, RX218, start=True, stop=True)
    K_dup = sing.tile([128, N], F32)
    nc.scalar.activation(out=K_dup, in_=zK, func=AF.Exp)
    nc.sync.dma_start(out=K_d[:, :], in_=K_dup[0:NL, :])
    K05 = sing.tile([128, N], F32)  # 0.5 * K
    nc.scalar.activation(out=K05, in_=K_dup, func=AF.Copy, scale=0.5)

    # ---- S_all: shat_b[i,j] = p_b[j] - p_b[i], dup halves, b-major ------
    # p_b rows are replicated across partitions with stride-0 SBUF->SBUF
    # DMAs (cheap, starts immediately after the MM18 load), then the
    # -p_b[i] bias is applied in place on the scalar engine.
    S_all = sing.tile([128, D * N], F32)
    for b in range(D):
        # DRAM-side row of P2 inside MM18, partition-broadcast (step 0)
        bcast = bass.AP(
            tensor=MM_d.tensor,
            offset=MM_d.offset + (b % 2) * MMW + C_P2 + (b // 2) * N,
            ap=[[0, 128], [1, N]])
        nc.sync.dma_start(out=S_all[:, b * N:(b + 1) * N], in_=bcast)
    for b in range(D):
        sl = S_all[:, b * N:(b + 1) * N]
        nc.scalar.activation(out=sl, in_=sl, func=AF.Identity,
                             bias=PL4N[:, b:b + 1])

    # ---- main hessian loop ---------------------------------------------
    for r in range(R):
        # pb = p_{2r}[j] on the upper 64 partitions, p_{2r+1}[j] on the lower
        pb = ps_b.tile([128, N], F32, tag="pbig")
        nc.tensor.matmul(pb, sel2, P2[:, r * N:(r + 1) * N],
                         start=True, stop=True)
        # T_r = shat_a * (-0.25*kNN)   (a = 2r upper half, 2r+1 lower half)
        T_r = tpool.tile([128, N], F32)
        nc.vector.scalar_tensor_tensor(
            out=T_r, in0=pb, scalar=PL3[:, r:r + 1], in1=kNNq,
            op0=OP.subtract, op1=OP.mult)

        # H_t[:, b*N:(b+1)*N] = T_r * S_all[b] for all 16 b, as one wide
        # DVE op (chunks 0:DVE_B) + one GPSIMD op (rest), broadcasting T_r
        # along the chunk axis with a stride-0 AP.
        H_t = hout.tile([128, D * N], F32)
        def tb(k):
            return bass.AP(tensor=T_r.tensor, offset=T_r.offset,
                           ap=[T_r.ap[0], [0, k], T_r.ap[1]])
        nD = DVE_B * N
        nc.vector.tensor_mul(
            H_t[:, 0:nD].rearrange("p (b j) -> p b j", b=DVE_B), tb(DVE_B),
            S_all[:, 0:nD].rearrange("p (b j) -> p b j", b=DVE_B))
        nc.gpsimd.tensor_mul(
            H_t[:, nD:D * N].rearrange("p (b j) -> p b j", b=D - DVE_B),
            tb(D - DVE_B),
            S_all[:, nD:D * N].rearrange("p (b j) -> p b j", b=D - DVE_B))
        # diagonal correction on the (a == b) blocks: Kc built on ACT,
        # then two in-place 64-partition adds on DVE
        Kc = tpool.tile([128, N], F32, tag="kc")
        nc.scalar.activation(out=Kc, in_=kNN, func=AF.Identity,
                             scale=CB2[:, r:r + 1])
        for half, b in ((0, 2 * r), (1, 2 * r + 1)):
            lo, hi = half * NL, half * NL + NL
            sl = H_t[lo:hi, b * N:(b + 1) * N]
            nc.vector.tensor_add(sl, sl, Kc[lo:hi, :])
        nc.sync.dma_start(out=H_d[r * 128:(r + 1) * 128, :], in_=H_t)

    # ---- grad_K ---------------------------------------------------------
    for tt in range(R):
        qb = ps_b.tile([128, N], F32, tag="pbig")
        nc.tensor.matmul(qb, sel2, Q2[:, tt * N:(tt + 1) * N],
                         start=True, stop=True)
        G_t = gout.tile([128, N], F32)
        nc.vector.scalar_tensor_tensor(
            out=G_t, in0=qb, scalar=PL3[:, tt:tt + 1], in1=K05,
            op0=OP.subtract, op1=OP.mult)
        nc.scalar.dma_start(out=G_d[tt * 128:(tt + 1) * 128, :], in_=G_t)


def build_nc():
    nc = bacc.Bacc()
    MM_d = nc.dram_tensor("MM18", [D + 2, MMW], F32,
                          kind="ExternalInput").ap()
    VT_d = nc.dram_tensor("VT", [128, VTW], F32, kind="ExternalInput").ap()
    K_d = nc.dram_tensor("Kout", [NL, N], F32, kind="ExternalOutput").ap()
    G_d = nc.dram_tensor("Gout", [NL * D, N], F32, kind="ExternalOutput").ap()
    H_d = nc.dram_tensor("Hout", [NL * D, N * D], F32,
                         kind="ExternalOutput").ap()
    with tile.TileContext(nc) as tc:
        with ExitStack() as ctx:
            _body(ctx, tc, nc, (MM_d, VT_d, K_d, G_d, H_d))
    # Bacc lowering: splits multi-sem waits into EventSemaphore instructions
    # (walrus allows at most one sync wait per engine instruction on TRN2),
    # moves matmul waits to ldweights, allocates registers.
    nc.compile()
    return nc


_CACHE = {}


def get_nc():
    if "nc" not in _CACHE:
        _CACHE["nc"] = build_nc()
    return _CACHE["nc"]


def make_in_maps(X, X2, uls, uvar):
    """Host prep: softplus the 17 hyperparameters and pack the small operand
    tables (O(N*D) f64 math) into MM18/VT; shard local-row tables per core."""
    X = np.asarray(X, np.float64)
    X2 = np.asarray(X2, np.float64)
    uls = np.asarray(uls, np.float64)
    uvar = np.asarray(uvar, np.float64)

    ls = np.logaddexp(0.0, uls)            # softplus
    var = np.logaddexp(0.0, uvar)[0]
    linv2 = 1.0 / (ls * ls)

    Xs = X / ls
    X2s = X2 / ls
    P = X * linv2                          # (N, D): p_d[x]
    Q = X2 * linv2                         # (N, D): q_d[m]
    nX = -0.25 * np.sum(Xs * Xs, axis=1)   # (N,)
    nX2 = -0.25 * np.sum(X2s * X2s, axis=1)

    mm = np.zeros((D + 2, MMW))
    mm[0:D, C_RX:C_RX + N] = 0.5 * Xs.T
    mm[D, C_RX:C_RX + N] = nX
    mm[D + 1, C_RX:C_RX + N] = 1.0
    mm[0:D, C_RX2:C_RX2 + N] = 0.5 * X2s.T
    mm[D, C_RX2:C_RX2 + N] = nX2
    mm[D + 1, C_RX2:C_RX2 + N] = 1.0
    mm[0, C_SEL:C_SEL + 64] = 1.0
    mm[1, C_SEL + 64:C_SEL + 128] = 1.0
    mm[0, C_ONE:C_ONE + 128] = 1.0
    mm[1, C_ONO:C_ONO + 128] = 1.0
    mm[0:2, C_P2:C_P2 + R * N] = \
        P.T.reshape(R, 2, N).transpose(1, 0, 2).reshape(2, R * N)
    mm[0:2, C_Q2:C_Q2 + R * N] = \
        Q.T.reshape(R, 2, N).transpose(1, 0, 2).reshape(2, R * N)

    vt = np.empty((128, VTW))
    vt[0:64, D + R:D + 2 * R] = (0.5 * linv2)[0::2][None, :]
    vt[64:128, D + R:D + 2 * R] = (0.5 * linv2)[1::2][None, :]

    maps = []
    for c in range(NCORES):
        rows = slice(c * NL, (c + 1) * NL)
        Xl = Xs[rows]                       # (64, D)
        Pl = P[rows]                        # (64, D)
        nl = -0.25 * np.sum(Xl * Xl, axis=1)
        mmc = mm.copy()
        mmc[0:D, C_L18:C_L18 + 64] = Xl.T
        mmc[0:D, C_L18 + 64:C_L18 + 128] = Xl.T
        mmc[D, C_L18:C_L18 + 128] = 1.0
        mmc[D + 1, C_L18:C_L18 + 64] = nl + np.log(var)
        mmc[D + 1, C_L18 + 64:C_L18 + 128] = nl + np.log(var)
        vtc = vt.copy()
        vtc[0:64, 0:D] = -Pl
        vtc[64:128, 0:D] = -Pl
        vtc[0:64, D:D + R] = Pl[:, 0::2]
        vtc[64:128, D:D + R] = Pl[:, 1::2]
        maps.append({
            "MM18": np.ascontiguousarray(mmc, dtype=np.float32),
            "VT": np.ascontiguousarray(vtc, dtype=np.float32),
        })
    return maps


def assemble(results):
    K = np.empty((N, N), np.float32)
    G = np.empty((N * D, N), np.float32)
    H = np.empty((N * D, N * D), np.float32)
    Gr = G.reshape(D, NCORES, NL, N)
    Hr = H.reshape(D, NCORES, NL, N * D)
    for c, res in enumerate(results):
        K[c * NL:(c + 1) * NL] = res["Kout"]
        Gr[:, c] = res["Gout"].reshape(D, NL, N)
        Hr[:, c] = res["Hout"].reshape(D, NL, N * D)
    return K, G, H


def run(X, X2, uls, uvar, trace=False, **kw):
    from concourse.bass_utils import run_bass_kernel_spmd

    nc = get_nc()
    in_maps = make_in_maps(X, X2, uls, uvar)
    out = run_bass_kernel_spmd(nc, in_maps, core_ids=list(range(NCORES)),
                               trace=trace, **kw)
    return assemble(out.results), out


def kernel(X, X2, uls, uvar):
    (K, G, H), _ = run(X, X2, uls, uvar)
    return K, G, H


# revision 20
# speedup vs baseline: 1.2849x; 1.1651x over previous
"""Trainium2 Bass kernel for DerivativeRBF: K(X,X2), grad_K, hess_K.

Math (reference):
  ls = softplus(uls) (D,), var = softplus(uvar)
  Xs = X/ls, X2s = X2/ls
  K[n,m]    = var*exp(-0.25*(|Xs_n|^2 - 2 Xs_n.X2s_m + |X2s_m|^2))      (N,M)
  grad_K    rows d*N+n: -0.5*(X[n,d]-X2[m,d])/ls_d^2 * K[n,m]           (N*D,M)
  kNN[i,j]  = var*exp(-0.25*sqdist(Xs_i,Xs_j))                          (N,N)
  hess_K[a*N+i, b*N+j] = kNN[i,j]*(delta_ab*0.5/ls_a^2
                                   - 0.25*s_a[i,j]*s_b[i,j])            (N*D,N*D)
  with s_d[i,j] = (X[i,d]-X[j,d])/ls_d^2.

Sharding: rows of X split across 8 cores, 64 rows each (SPMD, no
collectives). Each core computes its block-rows of all three outputs;
the host reassembles.

Host prep (tiny, O(N*D)): softplus of the 17 hyperparameters and the
scaled/transposed operand tables below. Device does all the heavy work:
the -0.25*sqdist matmuls + exp, and the D*D grid of N_loc x N hessian
blocks (8.4M f32 per core) plus grad blocks, via fused DVE/GPSIMD ops.

Device-side layout, per core (i0 = 64*core):
  p_d[x] = X[x,d]/ls_d^2;  shat_d[i,j] := p_d[j] - p_d[i] = -s_d[i,j]
  hess block (a,b) = (shat_a * (-0.25*kNN)) * shat_b  (+ delta_ab*c_a*kNN)
  row-pair tile r < 8: partitions 0:64 -> a=2r, 64:128 -> a=2r+1
"""

import sys

if "/opt/trn_rl_repo" not in sys.path:
    sys.path.insert(0, "/opt/trn_rl_repo")

from contextlib import ExitStack

import numpy as np

import concourse.bacc as bacc
import concourse.bass as bass
import concourse.tile as tile
from concourse import mybir

F32 = mybir.dt.float32
AF = mybir.ActivationFunctionType
OP = mybir.AluOpType

N = 512          # rows of X / X2
D = 16           # feature dim
NCORES = 8
NL = N // NCORES  # 64 local rows per core
R = D // 2        # 8 row-pair tiles (two feature dims per 128-partition tile)

# All matmul operands are packed into one [18, MMW] tensor (single DMA ->
# single completion semaphore; the fp32 fused-ldweights Matmult can encode
# only one sync wait, so every matmul may depend on at most one semaphore).
# Column layout of MM18:
#   0:128     L18    rows 0:16 Xs_loc.T dup; row 16 ones; row 17 -0.25*|Xs_loc|^2 dup
#   128:640   RX18   rows 0:16 0.5*Xs.T; row 16 -0.25*|Xs_j|^2; row 17 ones
#   640:1152  RX218  same with X2s
#   1152:1280 sel2   row 0: 1 on cols 0:64; row 1: 1 on cols 64:128
#   1280:1408 onesE  row 0 all ones, row 1 zeros
#   1408:1536 onesO  row 0 zeros, row 1 all ones
#   1536:5632 P2     row 0: p_{2r}[j] r-major; row 1: p_{2r+1}[j]
#   5632:9728 Q2     same from q_d[m] = X2[m,d]/ls_d^2
MMW = 9728
C_L18, C_RX, C_RX2, C_SEL, C_ONE, C_ONO, C_P2, C_Q2 = (
    0, 128, 640, 1152, 1280, 1408, 1536, 5632)
# Per-partition scalar tables (ACT bias / DVE scalar operands) in one
# [128, 32] tensor VT:
#   0:16  PL4N[p, d] = -p_d[i0 + p%64]        (bias for S_all builds)
#   16:24 PL3 col r: p_{2r}[i] upper half, p_{2r+1}[i] lower half
#   24:32 CBU col r: 0.5/ls_{2r}^2 upper half, 0 lower half
#   32:40 CBL col r: 0 upper half, 0.5/ls_{2r+1}^2 lower half
VTW = 40
GP_B = 5  # of the 14 non-diagonal chunks per r, the last GP_B go to GPSIMD


def _body(ctx, tc, nc, dram):
    MM_d, VT_d, K_d, G_d, H_d = dram

    sing = ctx.enter_context(tc.tile_pool(name="sing", bufs=1))
    ps_b = ctx.enter_context(tc.tile_pool(name="ps_b", bufs=4, space="PSUM"))
    tpool = ctx.enter_context(tc.tile_pool(name="tpool", bufs=2))
    hout = ctx.enter_context(tc.tile_pool(name="hout", bufs=2))
    gout = ctx.enter_context(tc.tile_pool(name="gout", bufs=2))

    MM = sing.tile([D + 2, MMW], F32)
    nc.sync.dma_start(out=MM, in_=MM_d[:, :])
    VT = sing.tile([128, VTW], F32)
    nc.sync.dma_start(out=VT, in_=VT_d[:, :])

    L18 = MM[:, C_L18:C_L18 + 128]
    RX18 = MM[:, C_RX:C_RX + N]
    RX218 = MM[:, C_RX2:C_RX2 + N]
    sel2 = MM[0:2, C_SEL:C_SEL + 128]
    onesE = MM[0:2, C_ONE:C_ONE + 128]
    onesO = MM[0:2, C_ONO:C_ONO + 128]
    P2 = MM[0:2, C_P2:C_P2 + R * N]
    Q2 = MM[0:2, C_Q2:C_Q2 + R * N]
    PL4N = VT[:, 0:D]
    PL3 = VT[:, D:D + R]
    CBU = VT[:, D + R:D + 2 * R]
    CBL = VT[:, D + 2 * R:D + 3 * R]

    # ---- kNN / K: z = -0.25*sqdist via one K=18 matmul each -------------
    zX = ps_b.tile([128, N], F32, tag="pbig")
    nc.tensor.matmul(zX, L18, RX18, start=True, stop=True)
    kNN = sing.tile([128, N], F32)
    nc.scalar.activation(out=kNN, in_=zX, func=AF.Exp)
    kNNq = sing.tile([128, N], F32)  # -0.25 * kNN
    nc.scalar.activation(out=kNNq, in_=kNN, func=AF.Copy, scale=-0.25)

    zK = ps_b.tile([128, N], F32, tag="pbig")
    nc.tensor.matmul(zK, L18, RX218, start=True, stop=True)
    K_dup = sing.tile([128, N], F32)
    nc.scalar.activation(out=K_dup, in_=zK, func=AF.Exp)
    nc.sync.dma_start(out=K_d[:, :], in_=K_dup[0:NL, :])
    K05 = sing.tile([128, N], F32)  # 0.5 * K
    nc.scalar.activation(out=K05, in_=K_dup, func=AF.Copy, scale=0.5)

    # ---- S_all: shat_b[i,j] = p_b[j] - p_b[i], dup halves, b-major ------
    # p_b rows are replicated across partitions with stride-0 SBUF->SBUF
    # DMAs (cheap, starts immediately after the MM18 load), then the
    # -p_b[i] bias is applied in place on the scalar engine.
    S_all = sing.tile([128, D * N], F32)
    for b in range(D):
        # DRAM-side row of P2 inside MM18, partition-broadcast (step 0)
        bcast = bass.AP(
            tensor=MM_d.tensor,
            offset=MM_d.offset + (b % 2) * MMW + C_P2 + (b // 2) * N,
            ap=[[0, 128], [1, N]])
        nc.sync.dma_start(out=S_all[:, b * N:(b + 1) * N], in_=bcast)
    # GPSIMD consumes the high chunks first; build those earliest
    for b in list(range(D - GP_B, D)) + list(range(D - GP_B)):
        sl = S_all[:, b * N:(b + 1) * N]
        nc.scalar.activation(out=sl, in_=sl, func=AF.Identity,
                             bias=PL4N[:, b:b + 1])

    # ---- main hessian loop ---------------------------------------------
    for r in range(R):
        # pb = p_{2r}[j] on the upper 64 partitions, p_{2r+1}[j] on the lower
        pb = ps_b.tile([128, N], F32, tag="pbig")
        nc.tensor.matmul(pb, sel2, P2[:, r * N:(r + 1) * N],
                         start=True, stop=True)
        # T_r = shat_a * (-0.25*kNN)   (a = 2r upper half, 2r+1 lower half)
        T_r = tpool.tile([128, N], F32)
        nc.vector.scalar_tensor_tensor(
            out=T_r, in0=pb, scalar=PL3[:, r:r + 1], in1=kNNq,
            op0=OP.subtract, op1=OP.mult)

        # H_t[:, b*N:(b+1)*N] = T_r * S_all[b], T_r broadcast along the
        # chunk axis with a stride-0 AP. The diagonal pair (b = 2r, 2r+1)
        # is handled separately; the remaining 14 chunks are split into
        # contiguous runs, the last GP_B chunks going to GPSIMD.
        H_t = hout.tile([128, D * N], F32)

        def tb(k):
            return bass.AP(tensor=T_r.tensor, offset=T_r.offset,
                           ap=[T_r.ap[0], [0, k], T_r.ap[1]])

        def fused_mul(eng, out_ap, lo, num):
            eng.tensor_mul(
                out_ap[:, lo * N:(lo + num) * N].rearrange(
                    "p (b j) -> p b j", b=num),
                tb(num),
                S_all[:, lo * N:(lo + num) * N].rearrange(
                    "p (b j) -> p b j", b=num))

        nondiag = [b for b in range(D) if b not in (2 * r, 2 * r + 1)]
        gp_set = set(nondiag[D - 2 - GP_B:])
        for eng, chunks in ((nc.vector, nondiag[:D - 2 - GP_B]),
                            (nc.gpsimd, nondiag[D - 2 - GP_B:])):
            run = []
            for b in chunks + [None]:
                if b is not None and (not run or b == run[-1] + 1):
                    run.append(b)
                    continue
                if run:
                    fused_mul(eng, H_t, run[0], len(run))
                run = [b]

        # diagonal pair: tmp = T*S, H = tmp + KcPair (KcPair is kNN scaled
        # by the half-masked diag constants; non-in-place add)
        KcP = tpool.tile([128, 2 * N], F32, tag="kcp")
        nc.scalar.activation(out=KcP[:, 0:N], in_=kNN, func=AF.Identity,
                             scale=CBU[:, r:r + 1])
        nc.scalar.activation(out=KcP[:, N:2 * N], in_=kNN, func=AF.Identity,
                             scale=CBL[:, r:r + 1])
        dtmp = tpool.tile([128, 2 * N], F32, tag="dtmp")
        nc.vector.tensor_mul(
            dtmp.rearrange("p (b j) -> p b j", b=2), tb(2),
            S_all[:, 2 * r * N:(2 * r + 2) * N].rearrange(
                "p (b j) -> p b j", b=2))
        nc.vector.tensor_add(H_t[:, 2 * r * N:(2 * r + 2) * N], dtmp, KcP)
        nc.sync.dma_start(out=H_d[r * 128:(r + 1) * 128, :], in_=H_t)

    # ---- grad_K ---------------------------------------------------------
    for tt in range(R):
        qb = ps_b.tile([128, N], F32, tag="pbig")
        nc.tensor.matmul(qb, sel2, Q2[:, tt * N:(tt + 1) * N],
                         start=True, stop=True)
        G_t = gout.tile([128, N], F32)
        nc.vector.scalar_tensor_tensor(
            out=G_t, in0=qb, scalar=PL3[:, tt:tt + 1], in1=K05,
            op0=OP.subtract, op1=OP.mult)
        nc.scalar.dma_start(out=G_d[tt * 128:(tt + 1) * 128, :], in_=G_t)


def build_nc():
    nc = bacc.Bacc()
    MM_d = nc.dram_tensor("MM18", [D + 2, MMW], F32,
                          kind="ExternalInput").ap()
    VT_d = nc.dram_tensor("VT", [128, VTW], F32, kind="ExternalInput").ap()
    K_d = nc.dram_tensor("Kout", [NL, N], F32, kind="ExternalOutput").ap()
    G_d = nc.dram_tensor("Gout", [NL * D, N], F32, kind="ExternalOutput").ap()
    H_d = nc.dram_tensor("Hout", [NL * D, N * D], F32,
                         kind="ExternalOutput").ap()
    with tile.TileContext(nc) as tc:
        with ExitStack() as ctx:
            _body(ctx, tc, nc, (MM_d, VT_d, K_d, G_d, H_d))
    # Bacc lowering: splits multi-sem waits into EventSemaphore instructions
    # (walrus allows at most one sync wait per engine instruction on TRN2),
    # moves matmul waits to ldweights, allocates registers.
    nc.compile()
    return nc


_CACHE = {}


def get_nc():
    if "nc" not in _CACHE:
        _CACHE["nc"] = build_nc()
    return _CACHE["nc"]


def make_in_maps(X, X2, uls, uvar):
    """Host prep: softplus the 17 hyperparameters and pack the small operand
    tables (O(N*D) f64 math) into MM18/VT; shard local-row tables per core."""
    X = np.asarray(X, np.float64)
    X2 = np.asarray(X2, np.float64)
    uls = np.asarray(uls, np.float64)
    uvar = np.asarray(uvar, np.float64)

    ls = np.logaddexp(0.0, uls)            # softplus
    var = np.logaddexp(0.0, uvar)[0]
    linv2 = 1.0 / (ls * ls)

    Xs = X / ls
    X2s = X2 / ls
    P = X * linv2                          # (N, D): p_d[x]
    Q = X2 * linv2                         # (N, D): q_d[m]
    nX = -0.25 * np.sum(Xs * Xs, axis=1)   # (N,)
    nX2 = -0.25 * np.sum(X2s * X2s, axis=1)

    mm = np.zeros((D + 2, MMW))
    mm[0:D, C_RX:C_RX + N] = 0.5 * Xs.T
    mm[D, C_RX:C_RX + N] = nX
    mm[D + 1, C_RX:C_RX + N] = 1.0
    mm[0:D, C_RX2:C_RX2 + N] = 0.5 * X2s.T
    mm[D, C_RX2:C_RX2 + N] = nX2
    mm[D + 1, C_RX2:C_RX2 + N] = 1.0
    mm[0, C_SEL:C_SEL + 64] = 1.0
    mm[1, C_SEL + 64:C_SEL + 128] = 1.0
    mm[0, C_ONE:C_ONE + 128] = 1.0
    mm[1, C_ONO:C_ONO + 128] = 1.0
    mm[0:2, C_P2:C_P2 + R * N] = \
        P.T.reshape(R, 2, N).transpose(1, 0, 2).reshape(2, R * N)
    mm[0:2, C_Q2:C_Q2 + R * N] = \
        Q.T.reshape(R, 2, N).transpose(1, 0, 2).reshape(2, R * N)

    vt = np.zeros((128, VTW))
    vt[0:64, D + R:D + 2 * R] = (0.5 * linv2)[0::2][None, :]
    vt[64:128, D + 2 * R:D + 3 * R] = (0.5 * linv2)[1::2][None, :]

    maps = []
    for c in range(NCORES):
        rows = slice(c * NL, (c + 1) * NL)
        Xl = Xs[rows]                       # (64, D)
        Pl = P[rows]                        # (64, D)
        nl = -0.25 * np.sum(Xl * Xl, axis=1)
        mmc = mm.copy()
        mmc[0:D, C_L18:C_L18 + 64] = Xl.T
        mmc[0:D, C_L18 + 64:C_L18 + 128] = Xl.T
        mmc[D, C_L18:C_L18 + 128] = 1.0
        mmc[D + 1, C_L18:C_L18 + 64] = nl + np.log(var)
        mmc[D + 1, C_L18 + 64:C_L18 + 128] = nl + np.log(var)
        vtc = vt.copy()
        vtc[0:64, 0:D] = -Pl
        vtc[64:128, 0:D] = -Pl
        vtc[0:64, D:D + R] = Pl[:, 0::2]
        vtc[64:128, D:D + R] = Pl[:, 1::2]
        maps.append({
            "MM18": np.ascontiguousarray(mmc, dtype=np.float32),
            "VT": np.ascontiguousarray(vtc, dtype=np.float32),
        })
    return maps


def assemble(results):
    K = np.empty((N, N), np.float32)
    G = np.empty((N * D, N), np.float32)
    H = np.empty((N * D, N * D), np.float32)
    Gr = G.reshape(D, NCORES, NL, N)
    Hr = H.reshape(D, NCORES, NL, N * D)
    for c, res in enumerate(results):
        K[c * NL:(c + 1) * NL] = res["Kout"]
        Gr[:, c] = res["Gout"].reshape(D, NL, N)
        Hr[:, c] = res["Hout"].reshape(D, NL, N * D)
    return K, G, H


def run(X, X2, uls, uvar, trace=False, **kw):
    from concourse.bass_utils import run_bass_kernel_spmd

    nc = get_nc()
    in_maps = make_in_maps(X, X2, uls, uvar)
    out = run_bass_kernel_spmd(nc, in_maps, core_ids=list(range(NCORES)),
                               trace=trace, **kw)
    return assemble(out.results), out


def kernel(X, X2, uls, uvar):
    (K, G, H), _ = run(X, X2, uls, uvar)
    return K, G, H
